# revision 1
# baseline (speedup 1.0000x reference)
"""CIN (xDeepFM Compressed Interaction Network) forward on 8 Trainium2 cores.

Pure data-parallel over batch. Each core computes:
  x1 = relu(einsum('bhd,bmd,shm->bsd', x0, x0, W1) + b1)
  x2 = relu(einsum('bhd,bmd,shm->bsd', x1, x0, W2) + b2)
  out = concat([x1.sum(d), x2.sum(d)], -1)

Device layout: features on partitions, n = (b_local, d) flattened on the free
dim. Interaction products z[(h,m),n] = a[h,n]*b[m,n] are built elementwise in
bf16; the (h,m)->s contraction runs on PE with fp32 PSUM accumulation.

Layer 1 packs the 741 strictly-lower symmetric (h<m) pairs into 6 K=128
chunks (two host-gathered operand arrays streamed in); the 39 diagonal
x0[h]^2 terms come from an ACT Square of the natural x0 tile.

Layer 2 needs x0 row m replicated across 128 partitions for each m. The
replication is split across engines to balance DMA vs PE vs ACT:
  - 23 m's: stride-0 replicated DMA read from DRAM (partition_broadcast AP).
  - 16 m's: K=39 one-hot selection matmul into PSUM + ACT copy to SBUF.
Products run on DVE (bf16 2x mode); matmuls accumulate 39-deep in PSUM.
The d-sums use a bf16 tree of 2x-mode tensor_tensor adds (cheaper than the
1x-only tensor_reduce). Tuned with TimelineSim: ~490us/core modeled.
"""
import sys

for _p in ("/opt/trn_rl_repo", "/root/.axon_site/_ro/trn_rl_repo"):
    if _p not in sys.path:
        sys.path.insert(0, _p)

import numpy as np
import ml_dtypes
from contextlib import ExitStack

import concourse.bacc as bacc
import concourse.tile as tile
import concourse.mybir as mybir
from concourse.bass_utils import run_bass_kernel_spmd

F32 = mybir.dt.float32
BF16 = mybir.dt.bfloat16
BF = ml_dtypes.bfloat16

B, M, D = 8192, 39, 16
S1 = S2 = 128
NCORES = 8
BC = B // NCORES          # 1024 batch rows per core
N = BC * D                # 16384 free-dim columns per core
NT = int(__import__('os').environ.get('NT', '2048'))  # columns per stream tile
NTILES = N // NT
NQ = NT // 512            # 512-wide matmul slices per stream tile

# layer-2 route assignment per m
PE_ROUTE = frozenset(int(i * 39 / 16) for i in range(16))   # 16 via PE sel-mm
PE_LIST = sorted(PE_ROUTE)
GP_SET = frozenset()
COPY_ENG = "act"

# strictly-lower symmetric (h < m) pair index lists; diagonal via ACT Square
PAIRS = [(h, m) for h in range(M) for m in range(h + 1, M)]
NPAIR = len(PAIRS)        # 780
NCHUNK = (NPAIR + 127) // 128  # 7
NPAD = NCHUNK * 128       # 896

_cache = {}
TREE_RED = True
ABLATE = set()  # sim ablations: nor, noab, nott2, nott1, nomm2, nomm1, nored, norelu


def _tree_reduce(nc, pool, dst, xsrc, t):
    """dst[128,128] f32 <- sum over innermost 16 of xsrc [128, 2048] bf16."""
    import concourse.mybir as mybir
    v = xsrc[:].rearrange("p (b d) -> p b d", d=16)
    nb = NT // 16
    s1 = pool.tile([128, nb, 8], BF16, tag="ts1", bufs=2)
    nc.vector.tensor_tensor(s1[:], v[:, :, 0:8], v[:, :, 8:16],
                            mybir.AluOpType.add)
    s2 = pool.tile([128, nb, 4], BF16, tag="ts2", bufs=2)
    nc.vector.tensor_tensor(s2[:], s1[:, :, 0:4], s1[:, :, 4:8],
                            mybir.AluOpType.add)
    s3 = pool.tile([128, nb, 2], BF16, tag="ts3", bufs=2)
    nc.vector.tensor_tensor(s3[:], s2[:, :, 0:2], s2[:, :, 2:4],
                            mybir.AluOpType.add)
    nc.vector.tensor_tensor(dst, s3[:, :, 0], s3[:, :, 1],
                            mybir.AluOpType.add)


def _build():
    nc = bacc.Bacc("TRN2", target_bir_lowering=False, debug=False,
                   num_devices=NCORES)
    x0_d = nc.dram_tensor("x0r", (M, N), BF16, kind="ExternalInput")
    xa_d = nc.dram_tensor("x0a", (NPAD, N), BF16, kind="ExternalInput")
    xb_d = nc.dram_tensor("x0b", (NPAD, N), BF16, kind="ExternalInput")
    w1_d = nc.dram_tensor("w1l", (NPAD, S1), BF16, kind="ExternalInput")
    wd_d = nc.dram_tensor("w1diag", (M, S1), BF16, kind="ExternalInput")
    w2_d = nc.dram_tensor("w2l", (S1, M, S2), BF16, kind="ExternalInput")
    b1_d = nc.dram_tensor("b1c", (S1, 1), F32, kind="ExternalInput")
    b2_d = nc.dram_tensor("b2c", (S2, 1), F32, kind="ExternalInput")
    id_d = nc.dram_tensor("ident", (128, 128), F32, kind="ExternalInput")
    npe = max(1, len(PE_ROUTE))
    sel_d = nc.dram_tensor("selm", (M, npe, 128), BF16, kind="ExternalInput")
    out_d = nc.dram_tensor("out", (BC, S1 + S2), F32, kind="ExternalOutput")

    with tile.TileContext(nc) as tc:
        with ExitStack() as ctx:
            const = ctx.enter_context(tc.tile_pool(name="const", bufs=1))
            ab = ctx.enter_context(tc.tile_pool(name="ab", bufs=int(__import__("os").environ.get("AB_BUFS", "5"))))
            zp = ctx.enter_context(tc.tile_pool(name="zp", bufs=int(__import__("os").environ.get("ZP_BUFS", "8"))))
            z2p = ctx.enter_context(tc.tile_pool(name="z2p", bufs=int(__import__("os").environ.get("Z2_BUFS", "4"))))
            rp = ctx.enter_context(tc.tile_pool(name="rp", bufs=int(__import__("os").environ.get("RP_BUFS", "8"))))
            xp = ctx.enter_context(tc.tile_pool(name="xp", bufs=int(__import__("os").environ.get("XP_BUFS", "2"))))
            op = ctx.enter_context(tc.tile_pool(name="op", bufs=2))
            accp = ctx.enter_context(tc.tile_pool(name="accp", bufs=6,
                                                  space="PSUM"))
            rps = ctx.enter_context(tc.tile_pool(name="rps", bufs=2,
                                                 space="PSUM"))

            w1t = const.tile([128, NCHUNK, S1], BF16)
            wdt = const.tile([M, S1], BF16)
            w2t = const.tile([S1, M, S2], BF16)
            b1t = const.tile([S1, 1], F32)
            b2t = const.tile([S2, 1], F32)
            idt = const.tile([128, 128], F32)
            selt = const.tile([M, npe, 128], BF16)
            if ABLATE:
                dumr = const.tile([128, NT], BF16)
                nc.sync.dma_start(dumr[:], xa_d[0:128, 0:NT])
            else:
                dumr = None
            p1t = const.tile([S1, BC], F32)
            p2t = const.tile([S2, BC], F32)
            nc.sync.dma_start(w1t[:], w1_d[:].rearrange("(c p) s -> p c s", p=128))
            nc.sync.dma_start(wdt[:], wd_d[:])
            nc.sync.dma_start(w2t[:], w2_d[:])
            nc.sync.dma_start(b1t[:], b1_d[:])
            nc.sync.dma_start(b2t[:], b2_d[:])
            nc.sync.dma_start(idt[:], id_d[:])
            nc.sync.dma_start(selt[:], sel_d[:])

            for t in range(NTILES):
                lo = t * NT
                # ---- layer 1: z1 chunks first, then q-major contraction
                x0t = ab.tile([M, NT], BF16, tag="x0t")
                nc.sync.dma_start(x0t[:], x0_d[:, lo:lo + NT])
                sqt = zp.tile([M, NT], BF16, tag="sq", bufs=2)
                nc.scalar.square(sqt[:], x0t[:])
                z1s = []
                for c in range(NCHUNK):
                    if "noab" in ABLATE:
                        a = b = dumr
                    else:
                        a = ab.tile([128, NT], BF16, tag="a")
                        b = ab.tile([128, NT], BF16, tag="b")
                        nc.sync.dma_start(a[:], xa_d[c * 128:(c + 1) * 128, lo:lo + NT])
                        nc.sync.dma_start(b[:], xb_d[c * 128:(c + 1) * 128, lo:lo + NT])
                    if "nott1" in ABLATE:
                        z1 = dumr
                    else:
                        z1 = zp.tile([128, NT], BF16, tag="z1", name=f"z1_{t}_{c}")
                        nc.vector.tensor_tensor(z1[:], a[:], b[:], mybir.AluOpType.mult)
                    z1s.append(z1)
                x1b = xp.tile([S1, NT], BF16, tag="x1")
                for q in range(NQ):
                    acc1 = accp.tile([128, 512], F32, tag="acc",
                                     name=f"acc1_{t}_{q}")
                    for c in range(NCHUNK):
                        nc.tensor.matmul(acc1[:], w1t[:, c, :],
                                         z1s[c][:, q * 512:(q + 1) * 512],
                                         start=(c == 0), stop=False)
                    nc.tensor.matmul(acc1[:], wdt[:],
                                     sqt[:, q * 512:(q + 1) * 512],
                                     start=False, stop=True)
                    nc.scalar.activation(x1b[:, q * 512:(q + 1) * 512], acc1[:],
                                         mybir.ActivationFunctionType.Relu,
                                         bias=b1t[:])
                if "nored" not in ABLATE:
                    if TREE_RED:
                        _tree_reduce(nc, zp,
                                     p1t[:, t * (NT // D):(t + 1) * (NT // D)],
                                     x1b, t)
                    else:
                        nc.vector.tensor_reduce(
                            p1t[:, t * (NT // D):(t + 1) * (NT // D)],
                            x1b[:].rearrange("p (b d) -> p b d", d=D),
                            mybir.AxisListType.X, mybir.AluOpType.add)

                # ---- layer 2: z2 = x1 * bcast(x0[m]) with mixed R routes
                gpz = {}
                for m in sorted(GP_SET):
                    rg = rp.tile([128, NT], BF16, tag="rgp", bufs=len(GP_SET) + 1)
                    nc.sync.dma_start(
                        rg[:], x0_d[m:m + 1, lo:lo + NT].partition_broadcast(128))
                    zg = z2p.tile([128, NT], BF16, tag="zgp",
                                  bufs=len(GP_SET) + 1, name=f"zgp_{t}_{m}")
                    nc.gpsimd.tensor_tensor(zg[:], x1b[:], rg[:],
                                            mybir.AluOpType.mult)
                    gpz[m] = zg
                acc2 = [accp.tile([128, 512], F32, tag="acc", name=f"acc2_{t}_{q}")
                        for q in range(NQ)]
                for m in range(M):
                    if m in GP_SET:
                        for q in range(NQ):
                            nc.tensor.matmul(acc2[q][:], w2t[:, m, :],
                                             gpz[m][:, q * 512:(q + 1) * 512],
                                             start=(m == 0), stop=(m == M - 1))
                        continue
                    r = rp.tile([128, NT], BF16, tag="r")
                    if "nor" in ABLATE:
                        r = dumr
                    elif m in PE_ROUTE:
                        j = PE_LIST.index(m)
                        for q in range(NQ):
                            rq = rps.tile([128, 512], F32, tag="rps",
                                          name=f"rps_{t}_{m}_{q}")
                            nc.tensor.matmul(rq[:], selt[:, j, :],
                                             x0t[:, q * 512:(q + 1) * 512])
                            if COPY_ENG == "act":
                                nc.scalar.copy(r[:, q * 512:(q + 1) * 512], rq[:])
                            else:
                                nc.vector.tensor_copy(
                                    r[:, q * 512:(q + 1) * 512], rq[:])
                    elif True:
                        nc.sync.dma_start(
                            r[:],
                            x0_d[m:m + 1, lo:lo + NT].partition_broadcast(128))
                    if "nott2" in ABLATE:
                        z2 = x1b
                    elif m in GP_SET:
                        z2 = gpz[m]
                    else:
                        z2 = z2p.tile([128, NT], BF16, tag="z2")
                        nc.vector.tensor_tensor(z2[:], x1b[:], r[:],
                                                mybir.AluOpType.mult)
                    if "nomm2" not in ABLATE:
                        for q in range(NQ):
                            nc.tensor.matmul(acc2[q][:], w2t[:, m, :],
                                             z2[:, q * 512:(q + 1) * 512],
                                             start=(m == 0), stop=(m == M - 1))
                x2b = xp.tile([S2, NT], BF16, tag="x2")
                for q in range(NQ):
                    nc.scalar.activation(x2b[:, q * 512:(q + 1) * 512], acc2[q][:],
                                         mybir.ActivationFunctionType.Relu,
                                         bias=b2t[:])
                if "nored" not in ABLATE:
                    if TREE_RED:
                        _tree_reduce(nc, zp,
                                     p2t[:, t * (NT // D):(t + 1) * (NT // D)],
                                     x2b, t)
                    else:
                        nc.vector.tensor_reduce(
                            p2t[:, t * (NT // D):(t + 1) * (NT // D)],
                            x2b[:].rearrange("p (b d) -> p b d", d=D),
                            mybir.AxisListType.X, mybir.AluOpType.add)

            # ---- epilogue: transpose [s, b] -> out[b, s]
            for t in range(BC // 128):
                for which, (pt, col) in enumerate(((p1t, 0), (p2t, S1))):
                    tp = accp.tile([128, 128], F32, tag="acc")
                    nc.tensor.transpose(tp[:], pt[:, t * 128:(t + 1) * 128], idt[:])
                    st = op.tile([128, 128], F32, tag="st")
                    nc.scalar.copy(st[:], tp[:])
                    nc.sync.dma_start(
                        out_d[t * 128:(t + 1) * 128, col:col + 128], st[:])
    nc.compile()
    return nc


def _prep_inputs(x0, W1, b1, W2, b2):
    # per-core feature-major layout: x0r[c][m, b*D + d]
    x0r = (x0.reshape(NCORES, BC, M, D).transpose(0, 2, 1, 3)
           .reshape(NCORES, M, N).astype(BF))
    hidx = np.array([p[0] for p in PAIRS])
    midx = np.array([p[1] for p in PAIRS])
    # folded symmetric weights: columns are strictly-lower pairs
    w1sym = np.empty((NPAD, S1), np.float32)
    w1sym[:NPAIR] = W1[:, hidx, midx].T + W1[:, midx, hidx].T
    w1sym[NPAIR:] = 0.0
    w1l = w1sym.astype(BF)
    w1diag = np.ascontiguousarray(
        W1[:, np.arange(M), np.arange(M)].T).astype(BF)
    w2l = np.ascontiguousarray(W2.transpose(1, 2, 0)).astype(BF)  # [h, m, s]
    b1c = np.ascontiguousarray(b1.reshape(S1, 1).astype(np.float32))
    b2c = np.ascontiguousarray(b2.reshape(S2, 1).astype(np.float32))
    ident = np.eye(128, dtype=np.float32)
    pe_list = sorted(PE_ROUTE)
    npe = max(1, len(pe_list))
    selm = np.zeros((M, npe, 128), BF)
    for j, m in enumerate(pe_list):
        selm[m, j, :] = 1.0

    in_maps = []
    for c in range(NCORES):
        xr = x0r[c]
        pad = np.zeros((NPAD - NPAIR, N), BF)
        in_maps.append({
            "x0r": np.ascontiguousarray(xr),
            "x0a": np.concatenate([xr[hidx], pad], 0),
            "x0b": np.concatenate([xr[midx], pad], 0),
            "w1l": w1l, "w1diag": w1diag, "w2l": w2l, "b1c": b1c, "b2c": b2c, "ident": ident,
            "selm": selm,
        })
    return in_maps


def _run(inputs, trace=False):
    if "nc" not in _cache:
        _cache["nc"] = _build()
    in_maps = _prep_inputs(inputs["x0"], inputs["W1"], inputs["b1"],
                           inputs["W2"], inputs["b2"])
    res = run_bass_kernel_spmd(_cache["nc"], in_maps, list(range(NCORES)),
                               trace=trace)
    out = np.concatenate([r["out"] for r in res.results], 0)
    return out.astype(np.float32), res


def kernel(x0, W1, b1, W2, b2):
    out, _ = _run({"x0": np.asarray(x0), "W1": np.asarray(W1),
                   "b1": np.asarray(b1), "W2": np.asarray(W2),
                   "b2": np.asarray(b2)})
    return out

